# revision 1
# baseline (speedup 1.0000x reference)
"""Trainium2 Bass kernel for causal multi-head attention (dense transformer).

Problem shapes (hardcoded): x [2,2048,1024], 16 heads x 64 head-dim.
Sharding: data-parallel over batch (2) x tensor-parallel over heads (4/core)
on 8 NeuronCores. Each core computes the partial output (sum over its 4
heads) for one batch element; the host sums the 4 partials per batch and
adds b_O.

Per-core kernel (all matmuls float32r: fp32-rounded operands streaming at
bf16 rate, ~2e-4 rel err vs fp32):
  - host passes x^T and pre-transposed weights, so no on-device transposes;
    inputs are spread across all 3 DMA-capable queues (gpsimd casting DMAs
    for weights, SP+ACT HW-DGE + DVE cast for x^T)
  - QKV projections run chunk-major (contraction-outer) so the PE starts on
    the first x^T chunk instead of waiting for the full 8.4MB
  - scores are computed as S^T[k,q] (k on partitions) with the contraction
    zero-padded from 64 to 128 rows: half-array (K=64) matmuls never warm
    the PE HAM clock gate and run at 1.2GHz forever; padded full-array
    matmuls reach 2.4GHz.  exp is fused with the PSUM->SBUF evacuation on
    ScalarE; the causal mask is a 0/1 multiply on the diagonal block (DVE)
  - AV uses V augmented with a ones column so the softmax denominator falls
    out of the same matmul; z^T is produced directly in out-proj layout;
    strips are software-pipelined at depth 4 so AV(s) is emitted after
    scores(s+4) and the PE never stalls on the exp
  - normalization: DVE copy of the denominator row to partition 0, custom
    fast reciprocal, gpsimd partition_broadcast, one tensor_tensor multiply
    (reciprocal_approx_fast and partition_broadcast silently misbehave on
    hardware with partition-base-64 sources, hence the copy)
"""

import sys

if "/opt/trn_rl_repo" not in sys.path:
    sys.path.insert(0, "/opt/trn_rl_repo")

import numpy as np

B, S, D = 2, 2048, 1024
H, DH = 16, 64
NCORES = 8
NH = 4            # heads per core
KCH = D // 128    # contraction chunks over model dim
NT = S // 128     # 128-row tiles over sequence
QC = S // 512     # 512-wide q chunks
P = 128
MASK_VAL = -30000.0

_CACHE = {}


def _build_nc(debug=False):
    import concourse.tile as tile
    from concourse import bacc, mybir

    f32 = mybir.dt.float32
    f32r = mybir.dt.float32r
    bf16 = mybir.dt.bfloat16
    Exp = mybir.ActivationFunctionType.Exp
    mult = mybir.AluOpType.mult

    nc = bacc.Bacc("TRN2", target_bir_lowering=False, debug=False,
                   num_devices=NCORES)

    xt_d = nc.dram_tensor("xt", [D, S], f32, kind="ExternalInput").ap()
    wq_d = nc.dram_tensor("wq", [P, KCH * NH * DH], f32, kind="ExternalInput").ap()
    wk_d = nc.dram_tensor("wk", [P, KCH * NH * DH], f32, kind="ExternalInput").ap()
    wv_d = nc.dram_tensor("wv", [P, KCH * NH * DH], f32, kind="ExternalInput").ap()
    wo_d = nc.dram_tensor("wo", [P, 2 * D], f32, kind="ExternalInput").ap()
    bq_d = nc.dram_tensor("bq", [1, NH * DH], f32, kind="ExternalInput").ap()
    bk_d = nc.dram_tensor("bk", [1, NH * DH], f32, kind="ExternalInput").ap()
    bv_d = nc.dram_tensor("bv", [1, NH * DH], f32, kind="ExternalInput").ap()
    ones_d = nc.dram_tensor("ones", [1, S], f32, kind="ExternalInput").ap()
    zeros_d = nc.dram_tensor("zeros", [1, S], f32, kind="ExternalInput").ap()
    vones_d = nc.dram_tensor("vones", [P, NT * NH], f32, kind="ExternalInput").ap()
    tri_d = nc.dram_tensor("tri", [P, P], f32, kind="ExternalInput").ap()
    trim_d = nc.dram_tensor("trim", [P, P], f32, kind="ExternalInput").ap()
    iden_d = nc.dram_tensor("iden", [P, P], f32, kind="ExternalInput").ap()
    out_d = nc.dram_tensor("out", [S, D], f32, kind="ExternalOutput").ap()
    dbg = {}
    if debug:
        dbg["qt"] = nc.dram_tensor("dbg_qt", [P, 2 * S], f32, kind="ExternalOutput").ap()
        dbg["kt"] = nc.dram_tensor("dbg_kt", [P, NH * S], f32, kind="ExternalOutput").ap()
        dbg["v"] = nc.dram_tensor("dbg_v", [P, NT * NH * (DH + 1)], f32, kind="ExternalOutput").ap()
        dbg["zn"] = nc.dram_tensor("dbg_zn", [P, 2 * S], f32, kind="ExternalOutput").ap()
        dbg["es"] = nc.dram_tensor("dbg_es", [P, 1024], f32, kind="ExternalOutput").ap()
        dbg["av"] = nc.dram_tensor("dbg_av", [DH + 1, QC * 512], f32, kind="ExternalOutput").ap()
        dbg["rd"] = nc.dram_tensor("dbg_rd", [1, QC * 512], f32, kind="ExternalOutput").ap()
        dbg["rdb"] = nc.dram_tensor("dbg_rdb", [64, QC * 512], f32, kind="ExternalOutput").ap()

    with tile.TileContext(nc) as tc:
        from contextlib import ExitStack

        with ExitStack() as ctx:
            persist = ctx.enter_context(tc.tile_pool(name="persist", bufs=1))

            QT = persist.tile([P, 2, S], f32r)
            KT = persist.tile([P, NH, S], f32r)
            V = persist.tile([P, NT, NH, DH + 1], f32r)
            ZN = persist.tile([P, 2, S], f32r)
            WQ = persist.tile([P, KCH, NH * DH], f32r)
            WK = persist.tile([P, KCH, NH * DH], f32r)
            WV = persist.tile([P, KCH, NH * DH], f32r)
            WO = persist.tile([P, 2, D], f32r)
            BQ = persist.tile([1, NH * DH], f32r)
            BK = persist.tile([1, NH * DH], f32r)
            BV = persist.tile([1, NH * DH], f32r)
            ONES = persist.tile([1, S], f32r)
            TRI = persist.tile([P, P], f32)
            IDEN = persist.tile([P, P], bf16)
            ZSRC = persist.tile([64, 512], f32)


            # ---- input DMAs (gpsimd casts fp32 -> float32r in flight) ----
            nc.sync.dma_start(TRI, tri_d)
            nc.gpsimd.dma_start(IDEN, iden_d)

            nc.gpsimd.dma_start(BQ, bq_d)
            nc.gpsimd.dma_start(BK, bk_d)
            nc.gpsimd.dma_start(BV, bv_d)
            nc.gpsimd.dma_start(ONES, ones_d)
            nc.gpsimd.dma_start(WQ.rearrange("p a b -> p (a b)"), wq_d)
            nc.gpsimd.dma_start(WK.rearrange("p a b -> p (a b)"), wk_d)
            nc.gpsimd.dma_start(WV.rearrange("p a b -> p (a b)"), wv_d)
            nc.gpsimd.dma_start(V[:, :, :, DH:DH + 1], vones_d)

            xtb_pool = tc.tile_pool(name="xtb", bufs=1)
            xtb_ctx = xtb_pool.__enter__()
            xta_pool = tc.tile_pool(name="xta", bufs=1)
            xta_ctx = xta_pool.__enter__()
            XTb = xtb_ctx.tile([P, KCH, 1024], f32r)
            XTa = xta_ctx.tile([P, KCH, 1024], f32r)
            with tc.tile_pool(name="stg", bufs=4) as stg_ctx:
                for ch in range(KCH - 3):
                    for hh in range(2):
                        stg = stg_ctx.tile([P, 1024], f32, tag="stg",
                                           name=f"stg_{ch}_{hh}")
                        eng = nc.sync if (ch + hh) % 2 == 0 else nc.scalar
                        eng.dma_start(stg, xt_d[ch * P:(ch + 1) * P,
                                                hh * 1024:(hh + 1) * 1024])
                        dst = XTa if hh == 0 else XTb
                        nc.vector.tensor_copy(dst[:, ch, :], stg)
                    if ch == 0:
                        nc.vector.memset(ZSRC, 0.0)
                        for h in range(NH):
                            zb = (h % 2) * 64 ^ 64
                            for qc in range(QC):
                                nc.vector.tensor_copy(
                                    KT[zb:zb + 64, h,
                                       qc * 512:(qc + 1) * 512], ZSRC)
                for ch in (KCH - 3, KCH - 2, KCH - 1):
                    for hh in range(2):
                        dst = XTa if hh == 0 else XTb
                        nc.gpsimd.dma_start(
                            dst[:, ch, :],
                            xt_d[ch * P:(ch + 1) * P,
                                 hh * 1024:(hh + 1) * 1024])
                nc.gpsimd.dma_start(WO.rearrange("p a b -> p (a b)"), wo_d)

                # ---- PE warmup while input DMAs stream ----
                with tc.tile_pool(name="warm_ps", bufs=1, space="PSUM") as wp:
                    wps = wp.tile([P, P], mybir.dt.float32)
                    for _ in range(36):
                        nc.tensor.matmul(wps, IDEN, IDEN, start=True, stop=True)

                def xt_cols(ch, c0, c1):
                    if c1 <= 1024:
                        return XTa[:, ch, c0:c1]
                    return XTb[:, ch, c0 - 1024:c1 - 1024]

                def qk_sweep(qkv_ps, sweep):
                    pst = {}
                    for wi, (W_, B_) in enumerate(((WQ, BQ), (WK, BK))):
                        for t in range(2):
                            for qc in (2 * sweep, 2 * sweep + 1):
                                pst[(wi, t, qc)] = qkv_ps.tile(
                                    [P, 512], mybir.dt.float32, tag="qk",
                                    name=f"qk{sweep}_{wi}_{t}_{qc}")
                    for ch in range(KCH):
                        for wi, (W_, B_) in enumerate(((WQ, BQ), (WK, BK))):
                            for t in range(2):
                                for qc in (2 * sweep, 2 * sweep + 1):
                                    nc.tensor.matmul(
                                        pst[(wi, t, qc)],
                                        W_[:, ch, t * P:(t + 1) * P],
                                        xt_cols(ch, qc * 512, (qc + 1) * 512),
                                        start=(ch == 0), stop=False)
                    for wi, (W_, B_) in enumerate(((WQ, BQ), (WK, BK))):
                        for t in range(2):
                            for qc in (2 * sweep, 2 * sweep + 1):
                                ps = pst[(wi, t, qc)]
                                nc.tensor.matmul(
                                    ps, B_[:, t * P:(t + 1) * P],
                                    ONES[:, qc * 512:(qc + 1) * 512],
                                    start=False, stop=True)
                                sl = slice(qc * 512, (qc + 1) * 512)
                                if wi == 0:
                                    nc.vector.tensor_copy(QT[:, t, sl], ps)
                                else:
                                    nc.vector.tensor_copy(
                                        KT[0:64, 2 * t, sl], ps[0:64, :])
                                    nc.vector.tensor_copy(
                                        KT[64:128, 2 * t + 1, sl], ps[64:128, :])

                # ---- phase 1: Q/K projections + all of V ----
                with tc.tile_pool(name="qkv_ps", bufs=8, space="PSUM") as qkv_ps:
                    qk_sweep(qkv_ps, 0)
                    qk_sweep(qkv_ps, 1)
                    for vs in range(2):
                        psv = [qkv_ps.tile([P, 512], mybir.dt.float32, tag="qk",
                                           name=f"v_{vs}_{i}")
                               for i in range(KCH)]
                        for ch in range(KCH):
                            for i in range(KCH):
                                kt = vs * KCH + i
                                nc.tensor.matmul(
                                    psv[i][:, 0:NH * DH],
                                    xt_cols(ch, kt * P, (kt + 1) * P),
                                    WV[:, ch, :], start=(ch == 0), stop=False)
                        for i in range(KCH):
                            kt = vs * KCH + i
                            nc.tensor.matmul(
                                psv[i][:, 0:NH * DH],
                                ONES[:, kt * P:(kt + 1) * P], BV,
                                start=False, stop=True)
                            nc.vector.tensor_copy(
                                V[:, kt, :, 0:DH], psv[i][:, 0:NH * DH])

            xta_pool.__exit__(None, None, None)
            xtb_pool.__exit__(None, None, None)

            # ---- attention: hf0 strips after 1a; sweep qc23; hf1 strips ----
            with tc.tile_pool(name="esp", bufs=4) as esp, \
                    tc.tile_pool(name="nrm", bufs=4) as nrm:
                avs = {}

                def emit_scores(sc_ps, h, kb, hf):
                    t, pb = h // 2, (h % 2) * 64
                    k0 = kb * P
                    hstart = hf * 1024
                    qstart = max(k0, hstart)
                    strip_ps = sc_ps.tile([P, 1024], mybir.dt.float32,
                                          name=f"sps_{h}_{kb}_{hf}", tag="sps")
                    strip_sb = esp.tile([P, 1024], f32r,
                                        name=f"ssb_{h}_{kb}_{hf}", tag="ssb")
                    has_diag = k0 >= hstart
                    qpos = qstart
                    while qpos < hstart + 1024:
                        qnext = min(hstart + 1024, (qpos // 512 + 1) * 512)
                        nc.tensor.matmul(
                            strip_ps[:, qpos - hstart:qnext - hstart],
                            KT[:, h, k0:k0 + P],
                            QT[:, t, qpos:qnext],
                            start=True, stop=True)
                        qpos = qnext
                    nc.scalar.activation(
                        strip_sb[:, qstart - hstart:1024],
                        strip_ps[:, qstart - hstart:1024], Exp)
                    if has_diag:
                        dsl = slice(k0 - hstart, k0 - hstart + P)
                        nc.vector.tensor_tensor(
                            strip_sb[:, dsl], strip_sb[:, dsl], TRI, mult)
                    if debug and h == 0 and kb == 0 and hf == 0:
                        nc.gpsimd.dma_start(dbg["es"], strip_sb)
                    return strip_sb

                def emit_av(av_ps, h, kb, hf, strip_sb):
                    k0 = kb * P
                    hstart = hf * 1024
                    qstart = max(k0, hstart)
                    if kb == 0:
                        for qc in (2 * hf, 2 * hf + 1):
                            avs[(h, qc)] = av_ps.tile(
                                [DH + 1, 512], mybir.dt.float32,
                                tag="av", name=f"av_{h}_{qc}")
                    av = {qc: avs[(h, qc)] for qc in (2 * hf, 2 * hf + 1)}
                    qpos = qstart
                    while qpos < hstart + 1024:
                        qc = qpos // 512
                        qnext = min(hstart + 1024, (qc + 1) * 512)
                        done = kb == 4 * qc + 3
                        nc.tensor.matmul(
                            av[qc][:, qpos - qc * 512:qnext - qc * 512],
                            V[:, kb, h, :],
                            strip_sb[:, qpos - hstart:qnext - hstart],
                            start=(kb == 0), stop=done)
                        if done:
                            emit_norm(h, qc, av[qc])
                        qpos = qnext

                def emit_norm(h, qc, avq):
                    t, pb = h // 2, (h % 2) * 64
                    if debug and h == 0:
                        avc = nrm.tile([DH + 1, 512], mybir.dt.float32,
                                       tag="avc", name=f"avc_{qc}")
                        nc.vector.tensor_copy(avc, avq)
                        nc.sync.dma_start(
                            dbg["av"][:, qc * 512:(qc + 1) * 512], avc)
                    rd = nrm.tile([1, 512], mybir.dt.float32, tag="rd")
                    nc.vector.tensor_copy(rd, avq[DH:DH + 1, :])
                    rr = nrm.tile([1, 512], mybir.dt.float32, tag="rr")
                    nc.vector.reciprocal_approx_fast(out=rr, in_=rd)
                    rdb = nrm.tile([64, 512], mybir.dt.float32, tag="rdb")
                    nc.gpsimd.partition_broadcast(rdb, rr)
                    if debug and h == 0:
                        nc.sync.dma_start(
                            dbg["rd"][:, qc * 512:(qc + 1) * 512], rr)
                        nc.sync.dma_start(
                            dbg["rdb"][:, qc * 512:(qc + 1) * 512], rdb)
                    zslc = ZN[pb:pb + 64, t, qc * 512:(qc + 1) * 512]
                    nc.vector.tensor_tensor(zslc, avq[0:DH, :], rdb, mult)

                from collections import deque

                def run_strips(sc_ps, av_ps, ids):
                    pending = deque()
                    for sid in ids:
                        sb_tile = emit_scores(sc_ps, *sid)
                        pending.append((sid, sb_tile))
                        if len(pending) > 4:
                            psid, psb = pending.popleft()
                            emit_av(av_ps, *psid, psb)
                    while pending:
                        psid, psb = pending.popleft()
                        emit_av(av_ps, *psid, psb)

                with tc.tile_pool(name="sc_psA", bufs=2, space="PSUM") as scA, \
                        tc.tile_pool(name="av_psA", bufs=4, space="PSUM") as avA:
                    run_strips(scA, avA,
                               [(h, kb, hf) for h in range(NH)
                                for hf in range(2) for kb in range(NT)
                                if hf * 1024 + 1024 > kb * P])

            if debug:
                for nm, tl in (("qt", QT), ("kt", KT), ("v", V), ("zn", ZN)):
                    nc.gpsimd.dma_start(dbg[nm], tl.rearrange("p ... -> p (...)"))

            # ---- phase 3: output projection ----
            with tc.tile_pool(name="op_ps", bufs=3, space="PSUM") as op_ps, \
                    tc.tile_pool(name="osb", bufs=3) as osb:
                for qt in range(NT):
                    for dc in range(2):
                        ps = op_ps.tile([P, 512], mybir.dt.float32)
                        for t in range(2):
                            nc.tensor.matmul(
                                ps, ZN[:, t, qt * P:(qt + 1) * P],
                                WO[:, t, dc * 512:(dc + 1) * 512],
                                start=(t == 0), stop=(t == 1))
                        ob = osb.tile([P, 512], mybir.dt.float32)
                        if (qt + dc) % 2 == 0:
                            nc.scalar.copy(ob, ps)
                        else:
                            nc.vector.tensor_copy(ob, ps)
                        oeng = (nc.sync, nc.scalar, nc.gpsimd)[(2 * qt + dc) % 3]
                        oeng.dma_start(
                            out_d[qt * P:(qt + 1) * P, dc * 512:(dc + 1) * 512],
                            ob)

    nc.compile()
    return nc


def _get_nc(debug=False):
    key = ("nc", debug)
    if key not in _CACHE:
        _CACHE[key] = _build_nc(debug)
    return _CACHE[key]


def _host_inputs(x, W_Q, W_K, W_V, W_O, b_Q, b_K, b_V):
    """Build the 8 per-core input maps."""
    x = np.asarray(x, dtype=np.float32)
    scale = 1.0 / np.sqrt(np.float32(DH))
    ones = np.ones((1, S), dtype=np.float32)
    vones = np.ones((P, NT * NH), dtype=np.float32)
    tri = (np.arange(P)[:, None] <= np.arange(P)[None, :]).astype(np.float32)
    trim = np.where(np.arange(P)[:, None] <= np.arange(P)[None, :],
                    np.float32(0.0), np.float32(MASK_VAL)).astype(np.float32)
    iden = np.eye(P, dtype=np.float32)

    xts = [np.ascontiguousarray(x[b].T) for b in range(B)]

    in_maps = []
    for c in range(NCORES):
        b, hg = divmod(c, NCORES // B)
        h0 = NH * hg
        def chunked(a):   # [D, M] -> [128, KCH*M] with rows p, cols (ch, m)
            return np.ascontiguousarray(
                a.reshape(KCH, P, -1).transpose(1, 0, 2).reshape(P, -1))
        wq = chunked((np.asarray(W_Q[h0:h0 + NH], np.float32) * scale)
                     .reshape(NH * DH, D).T)
        wk = chunked(np.asarray(W_K[h0:h0 + NH], np.float32)
                     .reshape(NH * DH, D).T)
        wv = chunked(np.asarray(W_V[h0:h0 + NH], np.float32)
                     .reshape(NH * DH, D).T)
        wo_flat = np.asarray(W_O[h0:h0 + NH], np.float32) \
            .transpose(0, 2, 1).reshape(NH * DH, D)
        wo = np.ascontiguousarray(
            wo_flat.reshape(2, P, D).transpose(1, 0, 2).reshape(P, 2 * D))
        bq = (np.asarray(b_Q[h0:h0 + NH], np.float32) * scale).reshape(1, NH * DH)
        bk = np.asarray(b_K[h0:h0 + NH], np.float32).reshape(1, NH * DH)
        bv = np.asarray(b_V[h0:h0 + NH], np.float32).reshape(1, NH * DH)
        in_maps.append({
            "xt": xts[b], "wq": wq, "wk": wk, "wv": wv, "wo": wo,
            "zeros": np.zeros((1, S), np.float32),
            "bq": np.ascontiguousarray(bq), "bk": np.ascontiguousarray(bk),
            "bv": np.ascontiguousarray(bv), "ones": ones, "vones": vones,
            "tri": tri, "trim": trim, "iden": iden,
        })
    return in_maps


def run_spmd(in_maps, debug=False, **kwargs):
    from concourse import bass_utils
    nc = _get_nc(debug)
    return bass_utils.run_bass_kernel_spmd(
        nc, in_maps, core_ids=list(range(NCORES)), **kwargs)


def kernel(x, W_Q, W_K, W_V, W_O, b_Q, b_K, b_V, b_O):
    in_maps = _host_inputs(x, W_Q, W_K, W_V, W_O, b_Q, b_K, b_V)
    res = run_spmd(in_maps)
    parts = [res.results[c]["out"] for c in range(NCORES)]
    gpb = NCORES // B
    out = np.stack(
        [sum(parts[b * gpb + g] for g in range(gpb)) for b in range(B)], axis=0)
    out += np.asarray(b_O, np.float32)[None, None, :]
    return out.astype(np.float32)



# revision 7
# speedup vs baseline: 1.4710x; 1.4710x over previous
"""Trainium2 Bass kernel for causal multi-head attention (dense transformer).

Problem shapes (hardcoded): x [2,2048,1024], 16 heads x 64 head-dim.
Sharding: data-parallel over batch (2) x tensor-parallel over heads (4/core)
on 8 NeuronCores. Each core computes the partial output (sum over its 4
heads) for one batch element; the host sums the 4 partials per batch and
adds b_O.

v2: all-bf16 datapath (host pre-casts; ~1e-3 rel err vs the 2e-2 gate).
  - bf16 HW-DGE DMAs (no on-device casts), FWL halves LDWEIGHTS
  - heads packed in pairs on partitions (even head d0-63, odd d64-127);
    scores run as K=64 matmul PAIRS on row groups (0,0)/(64,0) via
    base-partition auto tile_position -> both heads' scores in the time
    of one zero-padded matmul.  AV (K=128) matmuls keep the HAM warm.
  - causal mask accumulated into score PSUM by an IDEN x TRIM matmul
    (exp(-30000+s)=0), keeping DVE out of the strip critical path
  - exp on ACT over [128, 2(heads), w] strip groups; strips for early
    q-chunks are computed during the projection phases so ACT starts
    ~11us into the kernel and stays saturated
  - V carries a trailing ones column so the softmax denominator falls
    out of the AV matmul; norm: DVE copy (base-64 psum row) -> fast
    reciprocal -> gpsimd partition_broadcast -> DVE multiply
  - out-projection + bf16 output DMA pipelined behind attention
"""

import sys

if "/opt/trn_rl_repo" not in sys.path:
    sys.path.insert(0, "/opt/trn_rl_repo")

import numpy as np
import ml_dtypes

BF = ml_dtypes.bfloat16
B, S, D = 2, 2048, 1024
H, DH = 16, 64
NCORES = 8
NH = 4            # heads per core (2 pairs)
KCH = D // 128    # contraction chunks over model dim
NT = S // 128     # 128-row k tiles
QC = S // 512     # 512-wide q chunks
P = 128
MASK_VAL = -30000.0

_CACHE = {}


def _build_nc():
    import concourse.tile as tile
    from concourse import bacc, mybir

    f32 = mybir.dt.float32
    bf = mybir.dt.bfloat16
    Exp = mybir.ActivationFunctionType.Exp
    Copy = mybir.ActivationFunctionType.Copy
    mult = mybir.AluOpType.mult

    nc = bacc.Bacc("TRN2", target_bir_lowering=False, debug=False,
                   num_devices=NCORES)

    xt_d = nc.dram_tensor("xt", [D, S], bf, kind="ExternalInput").ap()
    wq_d = nc.dram_tensor("wq", [P, KCH * NH * DH], bf, kind="ExternalInput").ap()
    wk_d = nc.dram_tensor("wk", [P, KCH * NH * DH], bf, kind="ExternalInput").ap()
    wv_d = nc.dram_tensor("wv", [P, KCH * NH * DH], bf, kind="ExternalInput").ap()
    wo_d = nc.dram_tensor("wo", [P, 2 * D], bf, kind="ExternalInput").ap()
    bq_d = nc.dram_tensor("bq", [P, 2], f32, kind="ExternalInput").ap()
    bk_d = nc.dram_tensor("bk", [P, 2], f32, kind="ExternalInput").ap()
    bv_d = nc.dram_tensor("bv", [1, NH * DH], bf, kind="ExternalInput").ap()
    ones_d = nc.dram_tensor("ones", [1, P], bf, kind="ExternalInput").ap()
    vones_d = nc.dram_tensor("vones", [P, NT * NH], bf, kind="ExternalInput").ap()
    trim_d = nc.dram_tensor("trim", [P, P], bf, kind="ExternalInput").ap()
    iden_d = nc.dram_tensor("iden", [P, P], bf, kind="ExternalInput").ap()
    out_d = nc.dram_tensor("out", [S, D], bf, kind="ExternalOutput").ap()

    with tile.TileContext(nc) as tc:
        from contextlib import ExitStack

        with ExitStack() as ctx:
            persist = ctx.enter_context(tc.tile_pool(name="persist", bufs=1))

            WQ = persist.tile([P, KCH, NH * DH], bf)
            WK = persist.tile([P, KCH, NH * DH], bf)
            WV = persist.tile([P, KCH, NH * DH], bf)
            WO = persist.tile([P, 2, D], bf)
            BQ = persist.tile([P, 2], f32)
            BK = persist.tile([P, 2], f32)
            BV = persist.tile([1, NH * DH], bf)
            ONES = persist.tile([1, P], bf)
            TRIM = persist.tile([P, P], bf)
            IDEN = persist.tile([P, P], bf)
            QT = persist.tile([P, 2, S], bf)
            KT = persist.tile([P, 2, S], bf)
            V = persist.tile([P, NT, NH, DH + 1], bf)
            ZN = persist.tile([P, 2, S], bf)

            xt_ctx = ctx.enter_context(tc.tile_pool(name="xt", bufs=1))
            XT = xt_ctx.tile([P, KCH, S], bf)

            # ---- input DMAs (all HW-DGE, no casting) ----
            nc.sync.dma_start(IDEN, iden_d)
            nc.scalar.dma_start(TRIM, trim_d)
            for ch in range(KCH):
                eng = nc.sync if ch % 2 == 0 else nc.scalar
                eng.dma_start(XT[:, ch, :], xt_d[ch * P:(ch + 1) * P, :])
            nc.gpsimd.dma_start(WQ.rearrange("p a b -> p (a b)"), wq_d)
            nc.gpsimd.dma_start(WK.rearrange("p a b -> p (a b)"), wk_d)
            nc.gpsimd.dma_start(BQ, bq_d)
            nc.gpsimd.dma_start(BK, bk_d)
            nc.gpsimd.dma_start(ONES, ones_d)
            nc.gpsimd.dma_start(V[:, :, :, DH:DH + 1], vones_d)
            nc.gpsimd.dma_start(WV.rearrange("p a b -> p (a b)"), wv_d)
            nc.gpsimd.dma_start(BV, bv_d)
            nc.gpsimd.dma_start(WO.rearrange("p a b -> p (a b)"), wo_d)

            # strip-group score PSUM (2 tiles x 2 banks) and exp'd strips
            # SBUF ring (32 x 2KB/partition)
            sc_pool = ctx.enter_context(
                tc.tile_pool(name="sc", bufs=2, space="PSUM"))
            sbr_pool = ctx.enter_context(tc.tile_pool(name="sbr", bufs=32))
            nrm_pool = ctx.enter_context(tc.tile_pool(name="nrm", bufs=2))

            strips = {}       # (qc, t, kb) -> sb tile

            def emit_scores(qc, t, kbs):
                """Pair-packed scores + mask + exp for strips (qc, t, kb)."""
                for kb in kbs:
                    off = max(0, kb - 4 * qc) * P
                    diag = kb >= 4 * qc
                    g = sc_pool.tile([P, 2, 512], mybir.dt.float32, tag="g",
                                     name=f"g_{qc}_{t}_{kb}")
                    for s, pb in ((0, 0), (1, 64)):
                        nc.tensor.matmul(
                            g[:, s, off:512],
                            KT[pb:pb + 64, t, kb * P:(kb + 1) * P],
                            QT[pb:pb + 64, t, qc * 512 + off:(qc + 1) * 512],
                            start=True, stop=not diag)
                    if diag:
                        for s in range(2):
                            nc.tensor.matmul(
                                g[:, s, off:off + P], IDEN, TRIM,
                                start=False, stop=True, skip_group_check=True)
                    sb = sbr_pool.tile([P, 2, 512], bf, tag="sb",
                                       name=f"sb_{qc}_{t}_{kb}")
                    nc.scalar.activation(sb[:, :, off:512], g[:, :, off:512],
                                         Exp)
                    strips[(qc, t, kb)] = sb

            def emit_av_norm(qc, t, av_pool):
                """AV accumulation + normalization for (qc, t)."""
                av = [av_pool.tile([DH + 1, 512], mybir.dt.float32, tag="av",
                                   name=f"av_{qc}_{t}_{s}") for s in range(2)]
                last = 4 * qc + 3
                for kb in range(4 * qc + 4):
                    off = max(0, kb - 4 * qc) * P
                    sb = strips.pop((qc, t, kb))
                    for s in range(2):
                        nc.tensor.matmul(
                            av[s][:, off:512],
                            V[:, kb, 2 * t + s, :],
                            sb[:, s, off:512],
                            start=(kb == 0), stop=(kb == last))
                for s in range(2):
                    rd = nrm_pool.tile([1, 512], mybir.dt.float32, tag="rd",
                                       name=f"rd_{qc}_{t}_{s}")
                    nc.vector.tensor_copy(rd, av[s][DH:DH + 1, :])
                    rr = nrm_pool.tile([1, 512], mybir.dt.float32, tag="rr",
                                       name=f"rr_{qc}_{t}_{s}")
                    nc.vector.reciprocal_approx_fast(out=rr, in_=rd)
                    rdb = nrm_pool.tile([64, 512], mybir.dt.float32, tag="rdb",
                                        name=f"rdb_{qc}_{t}_{s}")
                    nc.gpsimd.partition_broadcast(rdb, rr)
                    nc.vector.tensor_tensor(
                        ZN[s * 64:(s + 1) * 64, t, qc * 512:(qc + 1) * 512],
                        av[s][0:DH, :], rdb, mult)

            n_evac = [0]

            def emit_outproj(qc, op_pool, ob_pool):
                for qt in range(4 * qc, 4 * qc + 4):
                    for dc in range(2):
                        ps = op_pool.tile([P, 512], mybir.dt.float32, tag="op",
                                          name=f"op_{qt}_{dc}")
                        for t in range(2):
                            nc.tensor.matmul(
                                ps, ZN[:, t, qt * P:(qt + 1) * P],
                                WO[:, t, dc * 512:(dc + 1) * 512],
                                start=(t == 0), stop=(t == 1))
                        ob = ob_pool.tile([P, 512], bf, tag="ob",
                                          name=f"ob_{qt}_{dc}")
                        i = n_evac[0]
                        n_evac[0] += 1
                        if i % 2 == 0:
                            nc.scalar.activation(ob, ps, Copy)
                        else:
                            nc.vector.tensor_copy(ob, ps)
                        oeng = (nc.sync, nc.gpsimd)[i % 2]
                        oeng.dma_start(
                            out_d[qt * P:(qt + 1) * P,
                                  dc * 512:(dc + 1) * 512], ob)

            # ============ phase 1: QK sweeps (+ scores qc0, qc1) ============
            with tc.tile_pool(name="qk_ps", bufs=4, space="PSUM") as qk_ps:
                for qc in range(QC):
                    pst = {}
                    for wi in range(2):
                        for t in range(2):
                            pst[(wi, t)] = qk_ps.tile(
                                [P, 512], mybir.dt.float32, tag="qk",
                                name=f"qk_{qc}_{wi}_{t}")
                    if qc == 0:
                        # PE warmup (HAM) while the first DMAs stream
                        for _ in range(36):
                            nc.tensor.matmul(pst[(0, 0)][:, 0:P], IDEN, IDEN,
                                             start=True, stop=True)
                    for ch in range(KCH):
                        for wi, W_ in ((0, WQ), (1, WK)):
                            for t in range(2):
                                nc.tensor.matmul(
                                    pst[(wi, t)],
                                    W_[:, ch, t * P:(t + 1) * P],
                                    XT[:, ch, qc * 512:(qc + 1) * 512],
                                    start=(ch == 0), stop=(ch == KCH - 1))
                    sl = slice(qc * 512, (qc + 1) * 512)
                    for t in range(2):
                        nc.vector.tensor_scalar_add(
                            QT[:, t, sl], pst[(0, t)], BQ[:, t:t + 1])
                        nc.vector.tensor_scalar_add(
                            KT[:, t, sl], pst[(1, t)], BK[:, t:t + 1])
                    if qc == 0:
                        for t in range(2):
                            emit_scores(0, t, range(4))
                    elif qc == 1:
                        for t in range(2):
                            emit_scores(1, t, range(8))

            # ===== phase 2: V proj + qc0/1 AV+norm + qc2 scores =====
            av_pool = ctx.enter_context(
                tc.tile_pool(name="av", bufs=2, space="PSUM"))

            with tc.tile_pool(name="v_ps", bufs=2, space="PSUM") as v_ps:
                def v_block(blk):
                    pv = v_ps.tile([P, 2, NH, DH], mybir.dt.float32, tag="v",
                                   name=f"v_{blk}")
                    for j in range(2):
                        kt = 2 * blk + j
                        for ch in range(KCH):
                            nc.tensor.matmul(
                                pv[:, j], XT[:, ch, kt * P:(kt + 1) * P],
                                WV[:, ch, :], start=(ch == 0), stop=False)
                        nc.tensor.matmul(
                            pv[:, j], ONES, BV, start=False, stop=True)
                        nc.vector.tensor_copy(V[:, kt, :, 0:DH], pv[:, j])

                # interleave V blocks between AV chains so the 2-deep av
                # ring never stalls the PE on norm latency
                v_block(0)
                v_block(1)
                emit_av_norm(0, 0, av_pool)
                v_block(2)
                emit_av_norm(0, 1, av_pool)
                v_block(3)
                emit_av_norm(1, 0, av_pool)
                qc2_sc = [(t, kb) for t in range(2) for kb in range(12)]
                v_block(4)
                for t, kb in qc2_sc[0:6]:
                    emit_scores(2, t, [kb])
                emit_av_norm(1, 1, av_pool)
                for blk in range(5, 8):
                    v_block(blk)
                    for t, kb in qc2_sc[(blk - 4) * 6:(blk - 3) * 6]:
                        emit_scores(2, t, [kb])

            # ============ phase 3: qc2/qc3 attention + outproj ============
            op_pool = ctx.enter_context(
                tc.tile_pool(name="op", bufs=2, space="PSUM"))
            ob_pool = ctx.enter_context(tc.tile_pool(name="ob", bufs=3))

            emit_outproj(0, op_pool, ob_pool)
            emit_av_norm(2, 0, av_pool)
            emit_outproj(1, op_pool, ob_pool)
            # qc3 t0 scores; interleave AV(2,1)
            for kb in range(8):
                emit_scores(3, 0, [kb])
            emit_av_norm(2, 1, av_pool)
            for kb in range(8, 16):
                emit_scores(3, 0, [kb])
            emit_av_norm(3, 0, av_pool)
            emit_outproj(2, op_pool, ob_pool)
            for kb in range(16):
                emit_scores(3, 1, [kb])
            emit_av_norm(3, 1, av_pool)
            emit_outproj(3, op_pool, ob_pool)

    nc.compile()
    return nc


def _get_nc():
    if "nc" not in _CACHE:
        _CACHE["nc"] = _build_nc()
    return _CACHE["nc"]


def _host_inputs(x, W_Q, W_K, W_V, W_O, b_Q, b_K, b_V):
    """Build the 8 per-core input maps (bf16 host-side preprocessing)."""
    x = np.asarray(x, dtype=np.float32)
    scale = 1.0 / np.sqrt(np.float32(DH))
    ones = np.ones((1, P), dtype=BF)
    vones = np.ones((P, NT * NH), dtype=BF)
    tri_mask = np.arange(P)[:, None] <= np.arange(P)[None, :]
    trim = np.where(tri_mask, np.float32(0.0),
                    np.float32(MASK_VAL)).astype(BF)
    iden = np.eye(P, dtype=BF)

    xts = [np.ascontiguousarray(x[b].T).astype(BF) for b in range(B)]

    in_maps = []
    for c in range(NCORES):
        b, hg = divmod(c, NCORES // B)
        h0 = NH * hg

        def chunked(a):   # [D, M] -> [128, KCH*M] rows p, cols (ch, m)
            return np.ascontiguousarray(
                a.reshape(KCH, P, -1).transpose(1, 0, 2).reshape(P, -1))

        wq = chunked((np.asarray(W_Q[h0:h0 + NH], np.float32) * scale)
                     .reshape(NH * DH, D).T).astype(BF)
        wk = chunked(np.asarray(W_K[h0:h0 + NH], np.float32)
                     .reshape(NH * DH, D).T).astype(BF)
        wv = chunked(np.asarray(W_V[h0:h0 + NH], np.float32)
                     .reshape(NH * DH, D).T).astype(BF)
        wo_flat = np.asarray(W_O[h0:h0 + NH], np.float32) \
            .transpose(0, 2, 1).reshape(NH * DH, D)
        wo = np.ascontiguousarray(
            wo_flat.reshape(2, P, D).transpose(1, 0, 2).reshape(P, 2 * D)) \
            .astype(BF)
        # per-partition bias columns: partition p of pair t = head
        # 2t + (p>=64), dh p%64
        bq_h = (np.asarray(b_Q[h0:h0 + NH], np.float32) * scale)
        bk_h = np.asarray(b_K[h0:h0 + NH], np.float32)
        bq = np.stack([bq_h[2 * t:2 * t + 2].reshape(P) for t in range(2)],
                      axis=1).astype(np.float32)
        bk = np.stack([bk_h[2 * t:2 * t + 2].reshape(P) for t in range(2)],
                      axis=1).astype(np.float32)
        bv = np.asarray(b_V[h0:h0 + NH], np.float32).reshape(1, NH * DH) \
            .astype(BF)
        in_maps.append({
            "xt": xts[b], "wq": wq, "wk": wk, "wv": wv, "wo": wo,
            "bq": np.ascontiguousarray(bq), "bk": np.ascontiguousarray(bk),
            "bv": np.ascontiguousarray(bv), "ones": ones, "vones": vones,
            "trim": trim, "iden": iden,
        })
    return in_maps


def run_spmd(in_maps, **kwargs):
    from concourse import bass_utils
    nc = _get_nc()
    return bass_utils.run_bass_kernel_spmd(
        nc, in_maps, core_ids=list(range(NCORES)), **kwargs)


def kernel(x, W_Q, W_K, W_V, W_O, b_Q, b_K, b_V, b_O):
    in_maps = _host_inputs(x, W_Q, W_K, W_V, W_O, b_Q, b_K, b_V)
    res = run_spmd(in_maps)
    gpb = NCORES // B
    parts = [np.asarray(res.results[c]["out"], dtype=np.float32)
             for c in range(NCORES)]
    out = np.stack(
        [sum(parts[b * gpb + g] for g in range(gpb)) for b in range(B)],
        axis=0)
    out += np.asarray(b_O, np.float32)[None, None, :]
    return out.astype(np.float32)


# revision 15
# speedup vs baseline: 1.4987x; 1.0188x over previous
"""Trainium2 Bass kernel for causal multi-head attention (dense transformer).

Problem shapes (hardcoded): x [2,2048,1024], 16 heads x 64 head-dim.
Sharding: data-parallel over batch (2) x tensor-parallel over heads (4/core)
on 8 NeuronCores. Each core computes the partial output (sum over its 4
heads) for one batch element; the host sums the 4 partials per batch and
adds b_O.

v2: all-bf16 datapath (host pre-casts; ~1e-3 rel err vs the 2e-2 gate).
  - bf16 HW-DGE DMAs (no on-device casts), FWL halves LDWEIGHTS
  - heads packed in pairs on partitions (even head d0-63, odd d64-127);
    scores run as K=64 matmul PAIRS on row groups (0,0)/(64,0) via
    base-partition auto tile_position -> both heads' scores in the time
    of one zero-padded matmul.  AV (K=128) matmuls keep the HAM warm.
  - causal mask accumulated into score PSUM by an IDEN x TRIM matmul
    (exp(-30000+s)=0), keeping DVE out of the strip critical path
  - exp on ACT over [128, 2(heads), w] strip groups; strips for early
    q-chunks are computed during the projection phases so ACT starts
    ~11us into the kernel and stays saturated
  - V carries a trailing ones column so the softmax denominator falls
    out of the AV matmul; norm: DVE copy (base-64 psum row) -> fast
    reciprocal -> gpsimd partition_broadcast -> DVE multiply
  - out-projection + bf16 output DMA pipelined behind attention
"""

import sys

if "/opt/trn_rl_repo" not in sys.path:
    sys.path.insert(0, "/opt/trn_rl_repo")

import numpy as np
import ml_dtypes

BF = ml_dtypes.bfloat16
B, S, D = 2, 2048, 1024
H, DH = 16, 64
NCORES = 8
NH = 4            # heads per core (2 pairs)
KCH = D // 128    # contraction chunks over model dim
NT = S // 128     # 128-row k tiles
QC = S // 512     # 512-wide q chunks
P = 128
MASK_VAL = -30000.0

_CACHE = {}


def _build_nc():
    import concourse.tile as tile
    from concourse import bacc, mybir

    f32 = mybir.dt.float32
    bf = mybir.dt.bfloat16
    Exp = mybir.ActivationFunctionType.Exp
    Copy = mybir.ActivationFunctionType.Copy
    mult = mybir.AluOpType.mult

    nc = bacc.Bacc("TRN2", target_bir_lowering=False, debug=False,
                   num_devices=NCORES)

    xt_d = nc.dram_tensor("xt", [D, S], bf, kind="ExternalInput").ap()
    wq_d = nc.dram_tensor("wq", [P, KCH * NH * DH], bf, kind="ExternalInput").ap()
    wk_d = nc.dram_tensor("wk", [P, KCH * NH * DH], bf, kind="ExternalInput").ap()
    wv_d = nc.dram_tensor("wv", [P, KCH * NH * DH], bf, kind="ExternalInput").ap()
    wo_d = nc.dram_tensor("wo", [P, 2 * D], bf, kind="ExternalInput").ap()
    bq_d = nc.dram_tensor("bq", [P, 2], f32, kind="ExternalInput").ap()
    bk_d = nc.dram_tensor("bk", [P, 2], f32, kind="ExternalInput").ap()
    bv_d = nc.dram_tensor("bv", [1, NH * DH], bf, kind="ExternalInput").ap()
    ones_d = nc.dram_tensor("ones", [1, P], bf, kind="ExternalInput").ap()
    vones_d = nc.dram_tensor("vones", [P, NT * NH], bf, kind="ExternalInput").ap()
    trim_d = nc.dram_tensor("trim", [P, P], bf, kind="ExternalInput").ap()
    iden_d = nc.dram_tensor("iden", [P, P], bf, kind="ExternalInput").ap()
    out_d = nc.dram_tensor("out", [S, D], bf, kind="ExternalOutput").ap()

    with tile.TileContext(nc) as tc:
        from contextlib import ExitStack

        with ExitStack() as ctx:
            persist = ctx.enter_context(tc.tile_pool(name="persist", bufs=1))

            WQ = persist.tile([P, KCH, NH * DH], bf)
            WK = persist.tile([P, KCH, NH * DH], bf)
            WV = persist.tile([P, KCH, NH * DH], bf)
            WO = persist.tile([P, 2, D], bf)
            BQ = persist.tile([P, 2], f32)
            BK = persist.tile([P, 2], f32)
            BV = persist.tile([1, NH * DH], bf)
            ONES = persist.tile([1, P], bf)
            TRIM = persist.tile([P, P], bf)
            IDEN = persist.tile([P, P], bf)
            QT = persist.tile([P, 2, S], bf)
            KT = persist.tile([P, 2, S], bf)
            V = persist.tile([P, NT, NH, DH + 1], bf)
            ZN = persist.tile([P, 2, S], bf)

            xt_ctx = ctx.enter_context(tc.tile_pool(name="xt", bufs=1))
            XT = xt_ctx.tile([P, KCH, S], bf)

            # ---- input DMAs (all HW-DGE, no casting), 4 parallel rings:
            # weights for the first sweeps ride ahead of the x chunks
            nc.sync.dma_start(IDEN, iden_d)
            nc.sync.dma_start(WQ.rearrange("p a b -> p (a b)"), wq_d)
            nc.scalar.dma_start(TRIM, trim_d)
            nc.scalar.dma_start(WK.rearrange("p a b -> p (a b)"), wk_d)
            qmap = {0: nc.gpsimd, 1: nc.sync, 2: nc.scalar, 3: nc.gpsimd,
                    4: nc.sync, 5: nc.scalar, 6: nc.gpsimd, 7: nc.sync}
            for ch in range(KCH):
                qmap[ch].dma_start(XT[:, ch, :], xt_d[ch * P:(ch + 1) * P, :])
            nc.gpsimd.dma_start(BQ, bq_d)
            nc.gpsimd.dma_start(BK, bk_d)
            nc.gpsimd.dma_start(ONES, ones_d)
            nc.gpsimd.dma_start(V[:, :, :, DH:DH + 1], vones_d)
            nc.gpsimd.dma_start(WV.rearrange("p a b -> p (a b)"), wv_d)
            nc.gpsimd.dma_start(BV, bv_d)
            nc.gpsimd.dma_start(WO.rearrange("p a b -> p (a b)"), wo_d)

            # strip-group score PSUM (2 tiles x 2 banks) and exp'd strips
            # SBUF ring (32 x 2KB/partition)
            sc_pool = ctx.enter_context(
                tc.tile_pool(name="sc", bufs=2, space="PSUM"))
            sbr_pool = ctx.enter_context(tc.tile_pool(name="sbr", bufs=32))
            nrm_pool = ctx.enter_context(tc.tile_pool(name="nrm", bufs=2))

            strips = {}       # (qc, t, kb) -> sb tile

            def emit_scores(qc, t, kbs):
                """Pair-packed scores + mask + exp for strips (qc, t, kb)."""
                for kb in kbs:
                    off = max(0, kb - 4 * qc) * P
                    diag = kb >= 4 * qc
                    g = sc_pool.tile([P, 2, 512], mybir.dt.float32, tag="g",
                                     name=f"g_{qc}_{t}_{kb}")
                    for s, pb in ((0, 0), (1, 64)):
                        nc.tensor.matmul(
                            g[:, s, off:512],
                            KT[pb:pb + 64, t, kb * P:(kb + 1) * P],
                            QT[pb:pb + 64, t, qc * 512 + off:(qc + 1) * 512],
                            start=True, stop=not diag)
                    if diag:
                        for s in range(2):
                            nc.tensor.matmul(
                                g[:, s, off:off + P], IDEN, TRIM,
                                start=False, stop=True, skip_group_check=True)
                    sb = sbr_pool.tile([P, 2, 512], bf, tag="sb",
                                       name=f"sb_{qc}_{t}_{kb}")
                    nc.scalar.activation(sb[:, :, off:512], g[:, :, off:512],
                                         Exp)
                    strips[(qc, t, kb)] = sb

            def g_av_norm(qc, t, av_pool, chunk=4):
                """AV accumulation + normalization for (qc, t); yields every
                `chunk` kb strips so callers can interleave PE work."""
                av = [av_pool.tile([DH + 1, 512], mybir.dt.float32, tag="av",
                                   name=f"av_{qc}_{t}_{s}") for s in range(2)]
                last = 4 * qc + 3
                for kb in range(4 * qc + 4):
                    off = max(0, kb - 4 * qc) * P
                    sb = strips.pop((qc, t, kb))
                    for s in range(2):
                        nc.tensor.matmul(
                            av[s][:, off:512],
                            V[:, kb, 2 * t + s, :],
                            sb[:, s, off:512],
                            start=(kb == 0), stop=(kb == last))
                    if kb % chunk == chunk - 1 and kb != last:
                        yield
                for s in range(2):
                    rd = nrm_pool.tile([1, 512], mybir.dt.float32, tag="rd",
                                       name=f"rd_{qc}_{t}_{s}")
                    nc.vector.tensor_copy(rd, av[s][DH:DH + 1, :])
                    rr = nrm_pool.tile([1, 512], mybir.dt.float32, tag="rr",
                                       name=f"rr_{qc}_{t}_{s}")
                    nc.vector.reciprocal_approx_fast(out=rr, in_=rd)
                    rdb = nrm_pool.tile([64, 512], mybir.dt.float32, tag="rdb",
                                        name=f"rdb_{qc}_{t}_{s}")
                    nc.gpsimd.partition_broadcast(rdb, rr)
                    nc.vector.tensor_tensor(
                        ZN[s * 64:(s + 1) * 64, t, qc * 512:(qc + 1) * 512],
                        av[s][0:DH, :], rdb, mult)
                yield

            def emit_av_norm(qc, t, av_pool):
                for _ in g_av_norm(qc, t, av_pool, chunk=99):
                    pass

            def g_outproj(qc, op_pool, ob_pool, act_evac=False):
                """Out-projection for q-chunk qc; yields per (qt, dc) tile.
                Evacuations go to DVE unless act_evac (tail: ACT is free)."""
                i = 0
                for qt in range(4 * qc, 4 * qc + 4):
                    for dc in range(2):
                        ps = op_pool.tile([P, 512], mybir.dt.float32, tag="op",
                                          name=f"op_{qt}_{dc}")
                        for t in range(2):
                            nc.tensor.matmul(
                                ps, ZN[:, t, qt * P:(qt + 1) * P],
                                WO[:, t, dc * 512:(dc + 1) * 512],
                                start=(t == 0), stop=(t == 1))
                        ob = ob_pool.tile([P, 512], bf, tag="ob",
                                          name=f"ob_{qt}_{dc}")
                        if act_evac and i % 2 == 0:
                            nc.scalar.activation(ob, ps, Copy)
                        else:
                            nc.vector.tensor_copy(ob, ps)
                        oeng = (nc.sync, nc.gpsimd)[i % 2]
                        oeng.dma_start(
                            out_d[qt * P:(qt + 1) * P,
                                  dc * 512:(dc + 1) * 512], ob)
                        i += 1
                        yield

            def emit_outproj(qc, op_pool, ob_pool):
                for _ in g_outproj(qc, op_pool, ob_pool):
                    pass

            def step(g, n=1):
                for _ in range(n):
                    if next(g, "end") == "end":
                        return

            # ============ phase 1: QK sweeps (+ scores qc0, qc1) ============
            with tc.tile_pool(name="qk_ps", bufs=4, space="PSUM") as qk_ps:
                for qc in range(QC):
                    pst = {}
                    for wi in range(2):
                        for t in range(2):
                            pst[(wi, t)] = qk_ps.tile(
                                [P, 512], mybir.dt.float32, tag="qk",
                                name=f"qk_{qc}_{wi}_{t}")
                    if qc == 0:
                        # PE warmup (HAM) while the first DMAs stream
                        for _ in range(36):
                            nc.tensor.matmul(pst[(0, 0)][:, 0:P], IDEN, IDEN,
                                             start=True, stop=True)
                    for ch in range(KCH):
                        for wi, W_ in ((0, WQ), (1, WK)):
                            for t in range(2):
                                nc.tensor.matmul(
                                    pst[(wi, t)],
                                    W_[:, ch, t * P:(t + 1) * P],
                                    XT[:, ch, qc * 512:(qc + 1) * 512],
                                    start=(ch == 0), stop=(ch == KCH - 1))
                    sl = slice(qc * 512, (qc + 1) * 512)
                    for t in range(2):
                        nc.vector.tensor_scalar_add(
                            QT[:, t, sl], pst[(0, t)], BQ[:, t:t + 1])
                        nc.vector.tensor_scalar_add(
                            KT[:, t, sl], pst[(1, t)], BK[:, t:t + 1])
                    if qc == 0:
                        for t in range(2):
                            emit_scores(0, t, range(4))
                    elif qc == 1:
                        for t in range(2):
                            emit_scores(1, t, range(8))

            # ===== phase 2: V proj + qc0/1 AV+norm + qc2 scores =====
            av_pool = ctx.enter_context(
                tc.tile_pool(name="av", bufs=2, space="PSUM"))

            with tc.tile_pool(name="v_ps", bufs=2, space="PSUM") as v_ps:
                def v_block(blk):
                    pv = v_ps.tile([P, 2, NH, DH], mybir.dt.float32, tag="v",
                                   name=f"v_{blk}")
                    for j in range(2):
                        kt = 2 * blk + j
                        for ch in range(KCH):
                            nc.tensor.matmul(
                                pv[:, j], XT[:, ch, kt * P:(kt + 1) * P],
                                WV[:, ch, :], start=(ch == 0), stop=False)
                        nc.tensor.matmul(
                            pv[:, j], ONES, BV, start=False, stop=True)
                        nc.vector.tensor_copy(V[:, kt, :, 0:DH], pv[:, j])

                # interleave V blocks between AV chains so the 2-deep av
                # ring never stalls the PE on norm latency
                v_block(0)
                v_block(1)
                emit_av_norm(0, 0, av_pool)
                v_block(2)
                emit_av_norm(0, 1, av_pool)
                v_block(3)
                emit_av_norm(1, 0, av_pool)
                qc2_sc = [(t, kb) for t in range(2) for kb in range(12)]
                v_block(4)
                av11 = g_av_norm(1, 1, av_pool)
                for i, (t, kb) in enumerate(qc2_sc[0:6]):
                    emit_scores(2, t, [kb])
                    if i % 2 == 1:
                        step(av11)
                v_block(5)
                for i, (t, kb) in enumerate(qc2_sc[6:12]):
                    emit_scores(2, t, [kb])
                    if i % 2 == 1:
                        step(av11)
                step(av11, 99)
                v_block(6)
                for t, kb in qc2_sc[12:18]:
                    emit_scores(2, t, [kb])
                v_block(7)
                for t, kb in qc2_sc[18:24]:
                    emit_scores(2, t, [kb])

            # ============ phase 3: qc2/qc3 attention + outproj ============
            # Score groups are emitted one at a time with small AV/outproj
            # quanta between them: the 2-deep score-PSUM ring means ACT can
            # only ever be 2 exps ahead, so long uninterrupted PE blocks
            # starve the exp stream.
            op_pool = ctx.enter_context(
                tc.tile_pool(name="op", bufs=2, space="PSUM"))
            ob_pool = ctx.enter_context(tc.tile_pool(name="ob", bufs=3))

            op0 = g_outproj(0, op_pool, ob_pool)
            op1 = g_outproj(1, op_pool, ob_pool)
            op2 = g_outproj(2, op_pool, ob_pool)
            op3 = g_outproj(3, op_pool, ob_pool, act_evac=True)
            av20 = g_av_norm(2, 0, av_pool)
            av21 = g_av_norm(2, 1, av_pool)
            av30 = g_av_norm(3, 0, av_pool)
            av31 = g_av_norm(3, 1, av_pool, chunk=2)

            # other-PE work to weave between qc3-t0 score groups
            weave0 = [op0, op0, op0, op0, av20, av20, av20, av20,
                      op1, op1, op1, op1, av21, av21, av21, av21]
            for kb in range(16):
                emit_scores(3, 0, [kb])
                step(weave0[kb])
            for g in (op0, op1, av20, av21):
                step(g, 99)
            weave1 = [av30, av30, av30, av30, op2, op2, op2, op2]
            for kb in range(8):
                emit_scores(3, 1, [kb])
                step(weave1[kb])
            for g in (av30, op2):
                step(g, 99)
            # tail: AV(3,1) chases its scores at a 2-kb lag
            for kb in range(8, 16):
                emit_scores(3, 1, [kb])
                if kb >= 10:
                    step(av31)
            step(av31, 99)
            step(op3, 99)

    nc.compile()
    return nc


def _get_nc():
    if "nc" not in _CACHE:
        _CACHE["nc"] = _build_nc()
    return _CACHE["nc"]


def _host_inputs(x, W_Q, W_K, W_V, W_O, b_Q, b_K, b_V):
    """Build the 8 per-core input maps (bf16 host-side preprocessing)."""
    x = np.asarray(x, dtype=np.float32)
    scale = 1.0 / np.sqrt(np.float32(DH))
    ones = np.ones((1, P), dtype=BF)
    vones = np.ones((P, NT * NH), dtype=BF)
    tri_mask = np.arange(P)[:, None] <= np.arange(P)[None, :]
    trim = np.where(tri_mask, np.float32(0.0),
                    np.float32(MASK_VAL)).astype(BF)
    iden = np.eye(P, dtype=BF)

    xts = [np.ascontiguousarray(x[b].T).astype(BF) for b in range(B)]

    in_maps = []
    for c in range(NCORES):
        b, hg = divmod(c, NCORES // B)
        h0 = NH * hg

        def chunked(a):   # [D, M] -> [128, KCH*M] rows p, cols (ch, m)
            return np.ascontiguousarray(
                a.reshape(KCH, P, -1).transpose(1, 0, 2).reshape(P, -1))

        wq = chunked((np.asarray(W_Q[h0:h0 + NH], np.float32) * scale)
                     .reshape(NH * DH, D).T).astype(BF)
        wk = chunked(np.asarray(W_K[h0:h0 + NH], np.float32)
                     .reshape(NH * DH, D).T).astype(BF)
        wv = chunked(np.asarray(W_V[h0:h0 + NH], np.float32)
                     .reshape(NH * DH, D).T).astype(BF)
        wo_flat = np.asarray(W_O[h0:h0 + NH], np.float32) \
            .transpose(0, 2, 1).reshape(NH * DH, D)
        wo = np.ascontiguousarray(
            wo_flat.reshape(2, P, D).transpose(1, 0, 2).reshape(P, 2 * D)) \
            .astype(BF)
        # per-partition bias columns: partition p of pair t = head
        # 2t + (p>=64), dh p%64
        bq_h = (np.asarray(b_Q[h0:h0 + NH], np.float32) * scale)
        bk_h = np.asarray(b_K[h0:h0 + NH], np.float32)
        bq = np.stack([bq_h[2 * t:2 * t + 2].reshape(P) for t in range(2)],
                      axis=1).astype(np.float32)
        bk = np.stack([bk_h[2 * t:2 * t + 2].reshape(P) for t in range(2)],
                      axis=1).astype(np.float32)
        bv = np.asarray(b_V[h0:h0 + NH], np.float32).reshape(1, NH * DH) \
            .astype(BF)
        in_maps.append({
            "xt": xts[b], "wq": wq, "wk": wk, "wv": wv, "wo": wo,
            "bq": np.ascontiguousarray(bq), "bk": np.ascontiguousarray(bk),
            "bv": np.ascontiguousarray(bv), "ones": ones, "vones": vones,
            "trim": trim, "iden": iden,
        })
    return in_maps


def run_spmd(in_maps, **kwargs):
    from concourse import bass_utils
    nc = _get_nc()
    return bass_utils.run_bass_kernel_spmd(
        nc, in_maps, core_ids=list(range(NCORES)), **kwargs)


def kernel(x, W_Q, W_K, W_V, W_O, b_Q, b_K, b_V, b_O):
    in_maps = _host_inputs(x, W_Q, W_K, W_V, W_O, b_Q, b_K, b_V)
    res = run_spmd(in_maps)
    gpb = NCORES // B
    parts = [np.asarray(res.results[c]["out"], dtype=np.float32)
             for c in range(NCORES)]
    out = np.stack(
        [sum(parts[b * gpb + g] for g in range(gpb)) for b in range(B)],
        axis=0)
    out += np.asarray(b_O, np.float32)[None, None, :]
    return out.astype(np.float32)


# revision 18
# speedup vs baseline: 1.5369x; 1.0255x over previous
"""Trainium2 Bass kernel for causal multi-head attention (dense transformer).

Problem shapes (hardcoded): x [2,2048,1024], 16 heads x 64 head-dim.
Sharding: data-parallel over batch (2) x tensor-parallel over heads (4/core)
on 8 NeuronCores. Each core computes the partial output (sum over its 4
heads) for one batch element; the host sums the 4 partials per batch and
adds b_O.

v2: all-bf16 datapath (host pre-casts; ~1e-3 rel err vs the 2e-2 gate).
  - bf16 HW-DGE DMAs (no on-device casts), FWL halves LDWEIGHTS
  - heads packed in pairs on partitions (even head d0-63, odd d64-127);
    scores run as K=64 matmul PAIRS on row groups (0,0)/(64,0) via
    base-partition auto tile_position -> both heads' scores in the time
    of one zero-padded matmul.  AV (K=128) matmuls keep the HAM warm.
  - causal mask accumulated into score PSUM by an IDEN x TRIM matmul
    (exp(-30000+s)=0), keeping DVE out of the strip critical path
  - exp on ACT over [128, 2(heads), w] strip groups; strips for early
    q-chunks are computed during the projection phases so ACT starts
    ~11us into the kernel and stays saturated
  - V carries a trailing ones column so the softmax denominator falls
    out of the AV matmul; norm: DVE copy (base-64 psum row) -> fast
    reciprocal -> gpsimd partition_broadcast -> DVE multiply
  - out-projection + bf16 output DMA pipelined behind attention
"""

import sys

if "/opt/trn_rl_repo" not in sys.path:
    sys.path.insert(0, "/opt/trn_rl_repo")

import numpy as np
import ml_dtypes

BF = ml_dtypes.bfloat16
B, S, D = 2, 2048, 1024
H, DH = 16, 64
NCORES = 8
NH = 4            # heads per core (2 pairs)
KCH = D // 128    # contraction chunks over model dim
NT = S // 128     # 128-row k tiles
QC = S // 512     # 512-wide q chunks
P = 128
MASK_VAL = -30000.0

_CACHE = {}


def _build_nc():
    import concourse.tile as tile
    from concourse import bacc, mybir

    f32 = mybir.dt.float32
    bf = mybir.dt.bfloat16
    Exp = mybir.ActivationFunctionType.Exp
    Copy = mybir.ActivationFunctionType.Copy
    mult = mybir.AluOpType.mult

    nc = bacc.Bacc("TRN2", target_bir_lowering=False, debug=False,
                   num_devices=NCORES)

    xt_d = nc.dram_tensor("xt", [D, S], bf, kind="ExternalInput").ap()
    wq_d = nc.dram_tensor("wq", [P, KCH * NH * DH], bf, kind="ExternalInput").ap()
    wk_d = nc.dram_tensor("wk", [P, KCH * NH * DH], bf, kind="ExternalInput").ap()
    wv_d = nc.dram_tensor("wv", [P, KCH * NH * DH], bf, kind="ExternalInput").ap()
    wo_d = nc.dram_tensor("wo", [P, 2 * D], bf, kind="ExternalInput").ap()
    bq_d = nc.dram_tensor("bq", [P, 2], f32, kind="ExternalInput").ap()
    bk_d = nc.dram_tensor("bk", [P, 2], f32, kind="ExternalInput").ap()
    bv_d = nc.dram_tensor("bv", [1, NH * DH], bf, kind="ExternalInput").ap()
    ones_d = nc.dram_tensor("ones", [1, P], bf, kind="ExternalInput").ap()
    vones_d = nc.dram_tensor("vones", [P, NT * NH], bf, kind="ExternalInput").ap()
    trim_d = nc.dram_tensor("trim", [P, P], bf, kind="ExternalInput").ap()
    iden_d = nc.dram_tensor("iden", [P, P], bf, kind="ExternalInput").ap()
    out_d = nc.dram_tensor("out", [S, D], bf, kind="ExternalOutput").ap()

    with tile.TileContext(nc) as tc:
        from contextlib import ExitStack

        with ExitStack() as ctx:
            persist = ctx.enter_context(tc.tile_pool(name="persist", bufs=1))

            WQ = persist.tile([P, KCH, NH * DH], bf)
            WK = persist.tile([P, KCH, NH * DH], bf)
            WV = persist.tile([P, KCH, NH * DH], bf)
            WO = persist.tile([P, 2, D], bf)
            BQ = persist.tile([P, 2], f32)
            BK = persist.tile([P, 2], f32)
            BV = persist.tile([1, NH * DH], bf)
            ONES = persist.tile([1, P], bf)
            TRIM = persist.tile([P, P], bf)
            IDEN = persist.tile([P, P], bf)
            QT = persist.tile([P, 2, S], bf)
            KT = persist.tile([P, 2, S], bf)
            V = persist.tile([P, NT, NH, DH + 1], bf)
            ZN = persist.tile([P, 2, S], bf)

            xt_ctx = ctx.enter_context(tc.tile_pool(name="xt", bufs=1))
            XT = xt_ctx.tile([P, KCH, S], bf)

            # ---- input DMAs (all HW-DGE, no casting), 4 parallel rings:
            # weights for the first sweeps ride ahead of the x chunks
            nc.sync.dma_start(IDEN, iden_d)
            nc.sync.dma_start(WQ.rearrange("p a b -> p (a b)"), wq_d)
            nc.scalar.dma_start(TRIM, trim_d)
            nc.scalar.dma_start(WK.rearrange("p a b -> p (a b)"), wk_d)
            # the gpsimd ring is the slowest (~90GB/s) -> 2 chunks only;
            # the scattered vones write goes last (not needed before AV)
            qmap = {0: nc.sync, 1: nc.scalar, 2: nc.gpsimd, 3: nc.sync,
                    4: nc.scalar, 5: nc.gpsimd, 6: nc.sync, 7: nc.scalar}
            for ch in range(KCH):
                qmap[ch].dma_start(XT[:, ch, :], xt_d[ch * P:(ch + 1) * P, :])
            nc.gpsimd.dma_start(BQ, bq_d)
            nc.gpsimd.dma_start(BK, bk_d)
            nc.gpsimd.dma_start(ONES, ones_d)
            nc.gpsimd.dma_start(WV.rearrange("p a b -> p (a b)"), wv_d)
            nc.gpsimd.dma_start(BV, bv_d)
            nc.gpsimd.dma_start(WO.rearrange("p a b -> p (a b)"), wo_d)
            nc.gpsimd.dma_start(V[:, :, :, DH:DH + 1], vones_d)

            # strip-group score PSUM (2 tiles x 2 banks) and exp'd strips
            # SBUF ring (32 x 2KB/partition)
            sc_pool = ctx.enter_context(
                tc.tile_pool(name="sc", bufs=2, space="PSUM"))
            sbr_pool = ctx.enter_context(tc.tile_pool(name="sbr", bufs=44))
            nrm_pool = ctx.enter_context(tc.tile_pool(name="nrm", bufs=2))

            strips = {}       # (qc, t, kb) -> sb tile

            def emit_scores(qc, t, kbs):
                """Pair-packed scores + mask + exp for strips (qc, t, kb)."""
                for kb in kbs:
                    off = max(0, kb - 4 * qc) * P
                    diag = kb >= 4 * qc
                    g = sc_pool.tile([P, 2, 512], mybir.dt.float32, tag="g",
                                     name=f"g_{qc}_{t}_{kb}")
                    for s, pb in ((0, 0), (1, 64)):
                        nc.tensor.matmul(
                            g[:, s, off:512],
                            KT[pb:pb + 64, t, kb * P:(kb + 1) * P],
                            QT[pb:pb + 64, t, qc * 512 + off:(qc + 1) * 512],
                            start=True, stop=not diag)
                    if diag:
                        for s in range(2):
                            nc.tensor.matmul(
                                g[:, s, off:off + P], IDEN, TRIM,
                                start=False, stop=True, skip_group_check=True)
                    sb = sbr_pool.tile([P, 2, 512], bf, tag="sb",
                                       name=f"sb_{qc}_{t}_{kb}")
                    nc.scalar.activation(sb[:, :, off:512], g[:, :, off:512],
                                         Exp)
                    strips[(qc, t, kb)] = sb

            def g_av_norm(qc, t, av_pool, chunk=4):
                """AV accumulation + normalization for (qc, t); yields every
                `chunk` kb strips so callers can interleave PE work."""
                av = [av_pool.tile([DH + 1, 512], mybir.dt.float32, tag="av",
                                   name=f"av_{qc}_{t}_{s}") for s in range(2)]
                last = 4 * qc + 3
                for kb in range(4 * qc + 4):
                    off = max(0, kb - 4 * qc) * P
                    sb = strips.pop((qc, t, kb))
                    for s in range(2):
                        nc.tensor.matmul(
                            av[s][:, off:512],
                            V[:, kb, 2 * t + s, :],
                            sb[:, s, off:512],
                            start=(kb == 0), stop=(kb == last))
                    if kb % chunk == chunk - 1 and kb != last:
                        yield
                for s in range(2):
                    rd = nrm_pool.tile([1, 512], mybir.dt.float32, tag="rd",
                                       name=f"rd_{qc}_{t}_{s}")
                    nc.vector.tensor_copy(rd, av[s][DH:DH + 1, :])
                    rr = nrm_pool.tile([1, 512], mybir.dt.float32, tag="rr",
                                       name=f"rr_{qc}_{t}_{s}")
                    nc.vector.reciprocal_approx_fast(out=rr, in_=rd)
                    rdb = nrm_pool.tile([64, 512], mybir.dt.float32, tag="rdb",
                                        name=f"rdb_{qc}_{t}_{s}")
                    nc.gpsimd.partition_broadcast(rdb, rr)
                    nc.vector.tensor_tensor(
                        ZN[s * 64:(s + 1) * 64, t, qc * 512:(qc + 1) * 512],
                        av[s][0:DH, :], rdb, mult)
                yield

            def emit_av_norm(qc, t, av_pool):
                for _ in g_av_norm(qc, t, av_pool, chunk=99):
                    pass

            def g_outproj(qc, op_pool, ob_pool, act_evac=False):
                """Out-projection for q-chunk qc; yields per (qt, dc) tile.
                Evacuations go to DVE unless act_evac (tail: ACT is free)."""
                i = 0
                for qt in range(4 * qc, 4 * qc + 4):
                    for dc in range(2):
                        ps = op_pool.tile([P, 512], mybir.dt.float32, tag="op",
                                          name=f"op_{qt}_{dc}")
                        for t in range(2):
                            nc.tensor.matmul(
                                ps, ZN[:, t, qt * P:(qt + 1) * P],
                                WO[:, t, dc * 512:(dc + 1) * 512],
                                start=(t == 0), stop=(t == 1))
                        ob = ob_pool.tile([P, 512], bf, tag="ob",
                                          name=f"ob_{qt}_{dc}")
                        if act_evac and i % 2 == 0:
                            nc.scalar.activation(ob, ps, Copy)
                        else:
                            nc.vector.tensor_copy(ob, ps)
                        oeng = (nc.sync, nc.gpsimd)[i % 2]
                        oeng.dma_start(
                            out_d[qt * P:(qt + 1) * P,
                                  dc * 512:(dc + 1) * 512], ob)
                        i += 1
                        yield

            def emit_outproj(qc, op_pool, ob_pool):
                for _ in g_outproj(qc, op_pool, ob_pool):
                    pass

            def step(g, n=1):
                for _ in range(n):
                    if next(g, "end") == "end":
                        return

            # ============ phase 1: QK sweeps (+ scores qc0, qc1) ============
            with tc.tile_pool(name="qk_ps", bufs=4, space="PSUM") as qk_ps:
                for qc in range(QC):
                    pst = {}
                    for wi in range(2):
                        for t in range(2):
                            pst[(wi, t)] = qk_ps.tile(
                                [P, 512], mybir.dt.float32, tag="qk",
                                name=f"qk_{qc}_{wi}_{t}")
                    if qc == 0:
                        # PE warmup (HAM) while the first DMAs stream
                        for _ in range(36):
                            nc.tensor.matmul(pst[(0, 0)][:, 0:P], IDEN, IDEN,
                                             start=True, stop=True)
                    for ch in range(KCH):
                        for wi, W_ in ((0, WQ), (1, WK)):
                            for t in range(2):
                                nc.tensor.matmul(
                                    pst[(wi, t)],
                                    W_[:, ch, t * P:(t + 1) * P],
                                    XT[:, ch, qc * 512:(qc + 1) * 512],
                                    start=(ch == 0), stop=(ch == KCH - 1))
                    sl = slice(qc * 512, (qc + 1) * 512)
                    for t in range(2):
                        nc.vector.tensor_scalar_add(
                            QT[:, t, sl], pst[(0, t)], BQ[:, t:t + 1])
                        nc.vector.tensor_scalar_add(
                            KT[:, t, sl], pst[(1, t)], BK[:, t:t + 1])
                    if qc == 0:
                        for t in range(2):
                            emit_scores(0, t, range(4))
                    elif qc == 1:
                        for t in range(2):
                            emit_scores(1, t, range(8))

            # ===== phase 2: V proj + qc0/1 AV+norm + qc2 scores =====
            av_pool = ctx.enter_context(
                tc.tile_pool(name="av", bufs=2, space="PSUM"))

            with tc.tile_pool(name="v_ps", bufs=2, space="PSUM") as v_ps:
                def v_block(blk):
                    pv = v_ps.tile([P, 2, NH, DH], mybir.dt.float32, tag="v",
                                   name=f"v_{blk}")
                    for j in range(2):
                        kt = 2 * blk + j
                        for ch in range(KCH):
                            nc.tensor.matmul(
                                pv[:, j], XT[:, ch, kt * P:(kt + 1) * P],
                                WV[:, ch, :], start=(ch == 0), stop=False)
                        nc.tensor.matmul(
                            pv[:, j], ONES, BV, start=False, stop=True)
                        nc.vector.tensor_copy(V[:, kt, :, 0:DH], pv[:, j])

                # interleave V blocks between AV chains so the 2-deep av
                # ring never stalls the PE on norm latency
                v_block(0)
                v_block(1)
                emit_av_norm(0, 0, av_pool)
                v_block(2)
                emit_av_norm(0, 1, av_pool)
                v_block(3)
                emit_av_norm(1, 0, av_pool)
                qc2_sc = [(t, kb) for t in range(2) for kb in range(12)]
                v_block(4)
                av11 = g_av_norm(1, 1, av_pool)
                for i, (t, kb) in enumerate(qc2_sc[0:6]):
                    emit_scores(2, t, [kb])
                    if i % 2 == 1:
                        step(av11)
                v_block(5)
                for i, (t, kb) in enumerate(qc2_sc[6:12]):
                    emit_scores(2, t, [kb])
                    if i % 2 == 1:
                        step(av11)
                step(av11, 99)
                v_block(6)
                for t, kb in qc2_sc[12:18]:
                    emit_scores(2, t, [kb])
                v_block(7)
                for t, kb in qc2_sc[18:24]:
                    emit_scores(2, t, [kb])

            # ============ phase 3: qc2/qc3 attention + outproj ============
            # Score groups are emitted one at a time with small AV/outproj
            # quanta between them: the 2-deep score-PSUM ring means ACT can
            # only ever be 2 exps ahead, so long uninterrupted PE blocks
            # starve the exp stream.
            op_pool = ctx.enter_context(
                tc.tile_pool(name="op", bufs=2, space="PSUM"))
            ob_pool = ctx.enter_context(tc.tile_pool(name="ob", bufs=3))

            op0 = g_outproj(0, op_pool, ob_pool)
            op1 = g_outproj(1, op_pool, ob_pool)
            op2 = g_outproj(2, op_pool, ob_pool)
            op3 = g_outproj(3, op_pool, ob_pool, act_evac=True)
            av20 = g_av_norm(2, 0, av_pool)
            av21 = g_av_norm(2, 1, av_pool)
            av30 = g_av_norm(3, 0, av_pool)
            av31 = g_av_norm(3, 1, av_pool, chunk=2)

            # other-PE work to weave between qc3-t0 score groups: ~2 quanta
            # per score group keeps PE-per-group ~= ACT exp time per group
            weave0 = [op0, av20, op0, av20, op0, av20, op0, av20,
                      op0, op1, op0, op1, op0, av21, op0, av21,
                      op1, av21, op1, av21, op1, op1, op1, op1,
                      op1, op1, op1, op1, op1, op1, op1, op1]
            for kb in range(16):
                emit_scores(3, 0, [kb])
                step(weave0[2 * kb])
                step(weave0[2 * kb + 1])
            for g in (op0, op1, av20, av21):
                step(g, 99)
            weave1 = [av30, op2, av30, op2, av30, op2, av30, op2,
                      op2, op2, op2, op2, op2, op2, op2, op2]
            for kb in range(8):
                emit_scores(3, 1, [kb])
                step(weave1[2 * kb])
                step(weave1[2 * kb + 1])
            for g in (av30, op2):
                step(g, 99)
            # tail: AV(3,1) chases its scores at a 2-kb lag
            for kb in range(8, 16):
                emit_scores(3, 1, [kb])
                if kb >= 10:
                    step(av31)
            step(av31, 99)
            step(op3, 99)

    nc.compile()
    return nc


def _get_nc():
    if "nc" not in _CACHE:
        _CACHE["nc"] = _build_nc()
    return _CACHE["nc"]


def _host_inputs(x, W_Q, W_K, W_V, W_O, b_Q, b_K, b_V):
    """Build the 8 per-core input maps (bf16 host-side preprocessing)."""
    x = np.asarray(x, dtype=np.float32)
    scale = 1.0 / np.sqrt(np.float32(DH))
    ones = np.ones((1, P), dtype=BF)
    vones = np.ones((P, NT * NH), dtype=BF)
    tri_mask = np.arange(P)[:, None] <= np.arange(P)[None, :]
    trim = np.where(tri_mask, np.float32(0.0),
                    np.float32(MASK_VAL)).astype(BF)
    iden = np.eye(P, dtype=BF)

    xts = [np.ascontiguousarray(x[b].T).astype(BF) for b in range(B)]

    in_maps = []
    for c in range(NCORES):
        b, hg = divmod(c, NCORES // B)
        h0 = NH * hg

        def chunked(a):   # [D, M] -> [128, KCH*M] rows p, cols (ch, m)
            return np.ascontiguousarray(
                a.reshape(KCH, P, -1).transpose(1, 0, 2).reshape(P, -1))

        wq = chunked((np.asarray(W_Q[h0:h0 + NH], np.float32) * scale)
                     .reshape(NH * DH, D).T).astype(BF)
        wk = chunked(np.asarray(W_K[h0:h0 + NH], np.float32)
                     .reshape(NH * DH, D).T).astype(BF)
        wv = chunked(np.asarray(W_V[h0:h0 + NH], np.float32)
                     .reshape(NH * DH, D).T).astype(BF)
        wo_flat = np.asarray(W_O[h0:h0 + NH], np.float32) \
            .transpose(0, 2, 1).reshape(NH * DH, D)
        wo = np.ascontiguousarray(
            wo_flat.reshape(2, P, D).transpose(1, 0, 2).reshape(P, 2 * D)) \
            .astype(BF)
        # per-partition bias columns: partition p of pair t = head
        # 2t + (p>=64), dh p%64
        bq_h = (np.asarray(b_Q[h0:h0 + NH], np.float32) * scale)
        bk_h = np.asarray(b_K[h0:h0 + NH], np.float32)
        bq = np.stack([bq_h[2 * t:2 * t + 2].reshape(P) for t in range(2)],
                      axis=1).astype(np.float32)
        bk = np.stack([bk_h[2 * t:2 * t + 2].reshape(P) for t in range(2)],
                      axis=1).astype(np.float32)
        bv = np.asarray(b_V[h0:h0 + NH], np.float32).reshape(1, NH * DH) \
            .astype(BF)
        in_maps.append({
            "xt": xts[b], "wq": wq, "wk": wk, "wv": wv, "wo": wo,
            "bq": np.ascontiguousarray(bq), "bk": np.ascontiguousarray(bk),
            "bv": np.ascontiguousarray(bv), "ones": ones, "vones": vones,
            "trim": trim, "iden": iden,
        })
    return in_maps


def run_spmd(in_maps, **kwargs):
    from concourse import bass_utils
    nc = _get_nc()
    return bass_utils.run_bass_kernel_spmd(
        nc, in_maps, core_ids=list(range(NCORES)), **kwargs)


def kernel(x, W_Q, W_K, W_V, W_O, b_Q, b_K, b_V, b_O):
    in_maps = _host_inputs(x, W_Q, W_K, W_V, W_O, b_Q, b_K, b_V)
    res = run_spmd(in_maps)
    gpb = NCORES // B
    parts = [np.asarray(res.results[c]["out"], dtype=np.float32)
             for c in range(NCORES)]
    out = np.stack(
        [sum(parts[b * gpb + g] for g in range(gpb)) for b in range(B)],
        axis=0)
    out += np.asarray(b_O, np.float32)[None, None, :]
    return out.astype(np.float32)
